# revision 1
# baseline (speedup 1.0000x reference)
"""Causal self-attention (GQA + RoPE) Trainium2 kernel.

Full-input contract: kernel(**inputs) takes the unsharded tensors and returns
the full [B, T, C] output. Internally shards over 8 NeuronCores as
(batch b in {0,1}) x (kv-head group g in {0..3}); each core computes the
attention output of its 4 query heads (one kv head) for its batch and the
partial out-projection against its 512 rows of Wo. The host sums the 4 group
partials per batch.

Per-core dataflow (all tensor-engine matmuls in float32r, fp32 PSUM accum):
  phase A: QT[d,t] = Wq_g^T x^T, KT, VT streamed over C-chunks; RoPE applied
           with host-precomputed transposed cos/sin tables (score scale folded
           into Wq, rotate-half sign folded into the sin table); V transposed
           to [t,d] via PE.
  phase B (per 512-query block): S^T[k,q] tiles on PE (F restricted to the
           causally-valid columns on diagonal tiles), triangle mask add (DVE),
           exp (ACT, PSUM->SBUF) with GpSimd zero-fill of the masked column
           range, denominator via all-ones matmul broadcast into PSUM,
           attn@V accumulated per head, normalization as a tensor-tensor
           divide, then the out-projection rows of this query block.
"""

import sys

for _p in ("/opt/trn_rl_repo", "/root/.axon_site/_ro/trn_rl_repo"):
    if _p not in sys.path:
        sys.path.append(_p)

import numpy as np
from contextlib import ExitStack

import concourse.bass as bass
import concourse.bacc as bacc
import concourse.tile as tile
import concourse.mybir as mybir
from concourse.bass_utils import run_bass_kernel_spmd

F32 = mybir.dt.float32
F32R = mybir.dt.float32r

B, T, C = 2, 2048, 2048
N_HEADS, N_KV_HEADS, HD = 16, 4, 128
G = N_HEADS // N_KV_HEADS  # heads per group = 4
GW = G * HD  # 512, per-group Q width / Wo row count
N_CORES = 8
TC = 512  # q-block width
NTC = T // TC  # 4
NKT = T // HD  # 16 k-tiles of 128
NCC = C // 128  # 16 contraction chunks
MASK_NEG = -1.0e30

_prog_cache = {}


def _build_program():
    nc = bacc.Bacc(
        "TRN2",
        target_bir_lowering=False,
        debug=False,
        enable_asserts=False,
        num_devices=N_CORES,
    )

    xT = nc.dram_tensor("xT", [C, T], F32, kind="ExternalInput").ap()
    wq = nc.dram_tensor("wq", [C, GW], F32, kind="ExternalInput").ap()
    wk = nc.dram_tensor("wk", [C, HD], F32, kind="ExternalInput").ap()
    wv = nc.dram_tensor("wv", [C, HD], F32, kind="ExternalInput").ap()
    wo = nc.dram_tensor("wo", [GW, C], F32, kind="ExternalInput").ap()
    cos = nc.dram_tensor("cos", [HD, T], F32, kind="ExternalInput").ap()
    sin = nc.dram_tensor("sin", [HD, T], F32, kind="ExternalInput").ap()
    masks = nc.dram_tensor("masks", [128, 128], F32, kind="ExternalInput").ap()
    ident = nc.dram_tensor("ident", [128, 128], F32, kind="ExternalInput").ap()
    onesfull = nc.dram_tensor("onesfull", [128, 128], F32, kind="ExternalInput").ap()
    y = nc.dram_tensor("y", [T, C], F32, kind="ExternalOutput").ap()

    with tile.TileContext(nc) as tc, ExitStack() as ctx:
        big_pool = ctx.enter_context(tc.tile_pool(name="big", bufs=1))

        # big activations: QT [d, h*T + t], KT [d, t], V [t-part, kt*HD + d]
        qt_sb = big_pool.tile([128, G * T], F32R)
        kt_sb = big_pool.tile([128, T], F32R)
        v_sb = big_pool.tile([128, NKT * HD], F32R)

        # ---------------- phase A: projections + rope ----------------
        with ExitStack() as pa:
            wpool = pa.enter_context(tc.tile_pool(name="wpool", bufs=1))
            xin = pa.enter_context(tc.tile_pool(name="xin", bufs=10))
            rp = pa.enter_context(tc.tile_pool(name="rp", bufs=3))
            qt_ps_pool = pa.enter_context(tc.tile_pool(name="qtps", bufs=4, space="PSUM"))
            warm_pool = pa.enter_context(tc.tile_pool(name="warm", bufs=1, space="PSUM"))
            kv_ps_pool = pa.enter_context(tc.tile_pool(name="kvps", bufs=2, space="PSUM"))
            tp_ps_pool = pa.enter_context(tc.tile_pool(name="tpps", bufs=1, space="PSUM"))

            # weight-chunk DMAs are interleaved into the first t-chunk's
            # c-loop so the first matmuls start as soon as chunk 0 lands
            wq_sb = wpool.tile([128, NCC * GW], F32R)  # [c-chunk p, ci*512 + j]
            wk_sb = wpool.tile([128, NCC * HD], F32R)
            wv_sb = wpool.tile([128, NCC * HD], F32R)
            cos_sb = wpool.tile([HD, T], F32)
            sin_sb = wpool.tile([HD, T], F32)
            ident_sb = wpool.tile([128, 128], F32)

            def load_w_chunk(ci):
                nc.sync.dma_start(
                    wq_sb[:, ci * GW : (ci + 1) * GW],
                    wq[ci * 128 : (ci + 1) * 128, :].bitcast(F32R),
                )
                nc.sync.dma_start(
                    wk_sb[:, ci * HD : (ci + 1) * HD],
                    wk[ci * 128 : (ci + 1) * 128, :].bitcast(F32R),
                )
                nc.sync.dma_start(
                    wv_sb[:, ci * HD : (ci + 1) * HD],
                    wv[ci * 128 : (ci + 1) * 128, :].bitcast(F32R),
                )

            for tci in range(NTC):
                ts = slice(tci * TC, (tci + 1) * TC)
                qt_ps = [
                    qt_ps_pool.tile([128, TC], F32, tag="qtps", name=f"qtps{tci}_{j}")
                    for j in range(G)
                ]
                kt_ps = kv_ps_pool.tile([128, TC], F32, tag="kvps", name=f"ktps{tci}")
                vt_ps = kv_ps_pool.tile([128, TC], F32, tag="kvps", name=f"vtps{tci}")
                for ci in range(NCC):
                    if tci == 0:
                        load_w_chunk(ci)
                        if ci == 10:
                            nc.gpsimd.dma_start(cos_sb[:], cos[:])
                            nc.gpsimd.dma_start(sin_sb[:], sin[:])
                            nc.gpsimd.dma_start(ident_sb[:], ident[:])
                    x_sb = xin.tile([128, TC], F32R, tag="x", name=f"x{tci}_{ci}")
                    xq = nc.gpsimd if (tci == 0 and ci % 2 == 1) else nc.sync
                    xq.dma_start(
                        x_sb[:], xT[ci * 128 : (ci + 1) * 128, ts].bitcast(F32R)
                    )
                    st, sp = (ci == 0), (ci == NCC - 1)
                    for j in range(G):
                        nc.tensor.matmul(
                            qt_ps[j][:],
                            wq_sb[:, ci * GW + j * HD : ci * GW + (j + 1) * HD],
                            x_sb[:],
                            start=st,
                            stop=sp,
                        )
                    nc.tensor.matmul(
                        kt_ps[:],
                        wk_sb[:, ci * HD : (ci + 1) * HD],
                        x_sb[:],
                        start=st,
                        stop=sp,
                    )
                    nc.tensor.matmul(
                        vt_ps[:],
                        wv_sb[:, ci * HD : (ci + 1) * HD],
                        x_sb[:],
                        start=st,
                        stop=sp,
                    )
                    # HAM warm-keeper: trivial matmul with no DMA dependency
                    # beyond the already-needed weight chunk keeps the PE
                    # activity window from idling during DMA-limited spans
                    wtile = warm_pool.tile([8, 8], F32, tag="warm", name=f"wm{tci}_{ci}")
                    nc.tensor.matmul(
                        wtile[:],
                        wq_sb[:, ci * GW : ci * GW + 8],
                        wq_sb[:, ci * GW : ci * GW + 8],
                        start=True,
                        stop=True,
                    )

                # rope on Q heads: out = q*cos + swap(q)*sin_signed.
                # The single ACT copy is the only psum reader, so the
                # accumulator bank frees as early as possible.
                for j in range(G):
                    q_raw = rp.tile([128, TC], F32, tag="qraw", name=f"qraw{tci}_{j}")
                    nc.scalar.copy(q_raw[:], qt_ps[j][:])
                    t1 = rp.tile([128, TC], F32, tag="t1", name=f"t1_{tci}_{j}")
                    nc.vector.tensor_mul(t1[:], q_raw[:], cos_sb[:, ts])
                    qsw = rp.tile([128, TC], F32, tag="qsw", name=f"qsw{tci}_{j}")
                    nc.gpsimd.dma_start(qsw[0:64, :], q_raw[64:128, :])
                    nc.gpsimd.dma_start(qsw[64:128, :], q_raw[0:64, :])
                    t2 = rp.tile([128, TC], F32, tag="t2", name=f"t2_{tci}_{j}")
                    nc.vector.tensor_mul(t2[:], qsw[:], sin_sb[:, ts])
                    nc.vector.tensor_add(
                        qt_sb[:, j * T + tci * TC : j * T + (tci + 1) * TC], t1[:], t2[:]
                    )
                # rope on K
                k_raw = rp.tile([128, TC], F32, tag="qraw", name=f"kraw{tci}")
                nc.scalar.copy(k_raw[:], kt_ps[:])
                t1k = rp.tile([128, TC], F32, tag="t1", name=f"t1k{tci}")
                nc.vector.tensor_mul(t1k[:], k_raw[:], cos_sb[:, ts])
                ksw = rp.tile([128, TC], F32, tag="qsw", name=f"ksw{tci}")
                nc.gpsimd.dma_start(ksw[0:64, :], k_raw[64:128, :])
                nc.gpsimd.dma_start(ksw[64:128, :], k_raw[0:64, :])
                t2k = rp.tile([128, TC], F32, tag="t2", name=f"t2k{tci}")
                nc.vector.tensor_mul(t2k[:], ksw[:], sin_sb[:, ts])
                nc.vector.tensor_add(kt_sb[:, ts], t1k[:], t2k[:])

                for wj in range(3):
                    wtile = warm_pool.tile(
                        [8, 8], F32, tag="warm", name=f"wmr{tci}_{wj}"
                    )
                    nc.tensor.matmul(
                        wtile[:], wq_sb[:, 0:8], wq_sb[:, 0:8], start=True, stop=True
                    )

                # V: [d, t] psum -> sbuf, then PE-transpose to [t, d]
                vt_f = rp.tile([128, TC], F32, tag="vtf", name=f"vtf{tci}")
                nc.scalar.copy(vt_f[:], vt_ps[:])
                for s in range(TC // 128):
                    kt_i = tci * (TC // 128) + s
                    tp_ps = tp_ps_pool.tile([128, 128], F32, tag="tp", name=f"tp{kt_i}")
                    nc.tensor.transpose(
                        tp_ps[:], vt_f[:, s * 128 : (s + 1) * 128], ident_sb[:]
                    )
                    nc.scalar.copy(v_sb[:, kt_i * HD : (kt_i + 1) * HD], tp_ps[:])

        # -------- phase B: attention + out-projection per q-block --------
        with ExitStack() as pb:
            st_pool = pb.enter_context(tc.tile_pool(name="stps", bufs=4, space="PSUM"))
            ot_ps_pool = pb.enter_context(tc.tile_pool(name="otps", bufs=2, space="PSUM"))
            s_ps_pool = pb.enter_context(tc.tile_pool(name="sps", bufs=2, space="PSUM"))
            pt_pool = pb.enter_context(tc.tile_pool(name="pt", bufs=10))
            nrm_pool = pb.enter_context(tc.tile_pool(name="nrm", bufs=3))
            ot_sb_pool = pb.enter_context(tc.tile_pool(name="otsb", bufs=2))
            y_sb_pool = pb.enter_context(tc.tile_pool(name="ysb", bufs=3))
            bconst = pb.enter_context(tc.tile_pool(name="bconst", bufs=1))

            mask_sb = bconst.tile([128, 128], F32)
            nc.sync.dma_start(mask_sb[:], masks[:])
            onesfull_sb = bconst.tile([128, 128], F32R)
            nc.sync.dma_start(onesfull_sb[:], onesfull.bitcast(F32R))
            wo_sb = bconst.tile([128, G * C], F32R)  # [j in head-chunk, h*C + c]
            for h in range(G):
                nc.sync.dma_start(
                    wo_sb[:, h * C : (h + 1) * C],
                    wo[h * 128 : (h + 1) * 128, :].bitcast(F32R),
                )

            for qb in range(NTC):
                nkt = (qb + 1) * (TC // 128)
                # ot: [d, h*TC + q] for this q-block
                ot_qb = ot_sb_pool.tile([128, G * TC], F32R, tag="ot", name=f"ot{qb}")
                for hg in range(G // 2):  # head pairs to fit PSUM
                    ot_ps = [
                        ot_ps_pool.tile(
                            [128, TC], F32, tag="otps", name=f"otps{qb}_{hg}_{hh}"
                        )
                        for hh in range(2)
                    ]
                    sb_ps = [
                        s_ps_pool.tile(
                            [128, TC], F32, tag="sps", name=f"sps{qb}_{hg}_{hh}"
                        )
                        for hh in range(2)
                    ]
                    for kt in range(nkt):
                        dj = kt - 4 * qb
                        f0 = max(dj, 0) * 128  # first causally-valid column
                        st, sp = (kt == 0), (kt == nkt - 1)
                        pts = []
                        for hh in range(2):
                            h = 2 * hg + hh
                            s_t = st_pool.tile(
                                [128, TC], F32, tag="st", name=f"st{qb}_{kt}_{h}"
                            )
                            nc.tensor.matmul(
                                s_t[:, f0:TC],
                                kt_sb[:, kt * 128 : (kt + 1) * 128],
                                qt_sb[:, h * T + qb * TC + f0 : h * T + (qb + 1) * TC],
                                start=True,
                                stop=True,
                            )
                            if dj >= 0:
                                nc.vector.tensor_add(
                                    s_t[:, f0 : f0 + 128],
                                    s_t[:, f0 : f0 + 128],
                                    mask_sb[:],
                                )
                            pt = pt_pool.tile(
                                [128, TC], F32R, tag="pt", name=f"pt{qb}_{kt}_{h}"
                            )
                            if f0 > 0:
                                nc.vector.memset(pt[:, 0:f0].bitcast(F32), 0.0)
                            nc.scalar.activation(
                                pt[:, f0:TC],
                                s_t[:, f0:TC],
                                mybir.ActivationFunctionType.Exp,
                            )
                            pts.append(pt)
                        for hh in range(2):
                            nc.tensor.matmul(
                                sb_ps[hh][:], onesfull_sb[:], pts[hh][:], start=st, stop=sp
                            )
                            nc.tensor.matmul(
                                ot_ps[hh][:],
                                v_sb[:, kt * HD : (kt + 1) * HD],
                                pts[hh][:],
                                start=st,
                                stop=sp,
                            )
                    for hh in range(2):
                        h = 2 * hg + hh
                        r_f = nrm_pool.tile([128, TC], F32, tag="rf", name=f"rf{qb}_{h}")
                        nc.vector.reciprocal_approx_fast(r_f[:], sb_ps[hh][:])
                        nc.vector.tensor_mul(
                            ot_qb[:, h * TC : (h + 1) * TC],
                            ot_ps[hh][:],
                            r_f[:],
                        )

                # out-projection for this q-block
                for tl in range(TC // 128):
                    tsub = qb * (TC // 128) + tl
                    for cc in range(C // TC):
                        y_ps = st_pool.tile(
                            [128, TC], F32, tag="st", name=f"yps{tsub}_{cc}"
                        )
                        for h in range(G):
                            nc.tensor.matmul(
                                y_ps[:],
                                ot_qb[:, h * TC + tl * 128 : h * TC + (tl + 1) * 128],
                                wo_sb[:, h * C + cc * TC : h * C + (cc + 1) * TC],
                                start=(h == 0),
                                stop=(h == G - 1),
                            )
                        y_sb = y_sb_pool.tile(
                            [128, TC], F32, tag="ysb", name=f"ysb{tsub}_{cc}"
                        )
                        nc.vector.tensor_copy(y_sb[:], y_ps[:])
                        nc.sync.dma_start(
                            y[tsub * 128 : (tsub + 1) * 128, cc * TC : (cc + 1) * TC],
                            y_sb[:],
                        )

    nc.compile()
    return nc


def _rope_tables():
    theta = 1.0 / (10000.0 ** (np.arange(0, HD, 2, dtype=np.float32) / HD))
    freqs = np.arange(T, dtype=np.float32)[:, None] * theta[None, :]  # [T, 64]
    cos = np.concatenate([np.cos(freqs), np.cos(freqs)], axis=-1)  # [T, 128]
    sin = np.concatenate([np.sin(freqs), np.sin(freqs)], axis=-1)
    cosT = np.ascontiguousarray(cos.T).astype(np.float32)  # [128, T]
    sinT = np.ascontiguousarray(sin.T).astype(np.float32)
    sign = np.where(np.arange(HD) < HD // 2, np.float32(-1.0), np.float32(1.0))[:, None]
    sinT_signed = (sinT * sign).astype(np.float32)
    return cosT, sinT_signed


def _masks():
    p = np.arange(128)[:, None]
    f = np.arange(128)[None, :]
    return np.where(p <= f, 0.0, MASK_NEG).astype(np.float32)


def make_in_maps(x, Wq, Wk, Wv, Wo):
    x = np.asarray(x, dtype=np.float32)
    Wq = np.asarray(Wq, dtype=np.float32)
    Wk = np.asarray(Wk, dtype=np.float32)
    Wv = np.asarray(Wv, dtype=np.float32)
    Wo = np.asarray(Wo, dtype=np.float32)

    cosT, sinT = _rope_tables()
    masks = _masks()
    qscale = np.float32(1.0 / np.sqrt(HD))
    ident = np.eye(128, dtype=np.float32)
    onesfull = np.ones((128, 128), dtype=np.float32)

    in_maps = []
    for c in range(N_CORES):
        b, g = divmod(c, N_KV_HEADS)
        in_maps.append(
            {
                "xT": np.ascontiguousarray(x[b].T),
                "wq": np.ascontiguousarray(Wq[:, g * GW : (g + 1) * GW]) * qscale,
                "wk": np.ascontiguousarray(Wk[:, g * HD : (g + 1) * HD]),
                "wv": np.ascontiguousarray(Wv[:, g * HD : (g + 1) * HD]),
                "wo": np.ascontiguousarray(Wo[g * GW : (g + 1) * GW, :]),
                "cos": cosT,
                "sin": sinT,
                "masks": masks,
                "ident": ident,
                "onesfull": onesfull,
            }
        )
    return in_maps


def kernel(x, Wq, Wk, Wv, Wo):
    if "nc" not in _prog_cache:
        _prog_cache["nc"] = _build_program()
    nc = _prog_cache["nc"]

    in_maps = make_in_maps(x, Wq, Wk, Wv, Wo)
    res = run_bass_kernel_spmd(nc, in_maps, list(range(N_CORES)))
    _prog_cache["last_results"] = res

    out = np.zeros((B, T, C), dtype=np.float32)
    for c in range(N_CORES):
        b = c // N_KV_HEADS
        out[b] += res.results[c]["y"]
    return out



# revision 4
# speedup vs baseline: 1.3505x; 1.3505x over previous
"""Causal self-attention (GQA + RoPE) Trainium2 kernel, bf16 tensor-core path.

Full-input contract: kernel(**inputs) takes the unsharded tensors and returns
the full [B, T, C] output. Internally shards over 8 NeuronCores as
(batch b in {0,1}) x (kv-head group g in {0..3}); each core computes the
attention output of its 4 query heads (one kv head) for its batch and the
partial out-projection against its 512 rows of Wo. The host sums the 4 group
partials per batch.

v2 design (vs the fp32r baseline):
  - all matmul operands bf16 (fp32 PSUM accumulation). Host converts inputs.
  - phase A (projections) and phase B (attention q-blocks) are interleaved
    chunk-wise in emission order so the PE never idles long enough to drop
    out of its ramped p-state: A0 A1 B0a A2 B0o B1a A3 B1o B2a B3a B2o B3o.
  - PSUM budget kept at 8 banks at every point: A accumulators 3 (two passes
    per t-chunk: {q0,q1,k} then {q2,q3,v}), scores/outproj 3, denom 1, attnV 1.
  - V is produced directly in [t, d] layout (x-chunk stationary, Wv moving),
    no PE transpose pass.
  - causal masking via a 0/1 lower-triangle multiply on the exp output (bf16,
    SBUF) instead of a -1e30 add on the fp32 PSUM scores; scores, exp, the
    denominator matmul and attn@V are all restricted to the causally valid
    column range [f0:TC] of each k-tile (accumulation regions only shrink
    after the full-width kt=0 tile, so partial-range PSUM accumulate is safe).
"""

import sys

for _p in ("/opt/trn_rl_repo", "/root/.axon_site/_ro/trn_rl_repo"):
    if _p not in sys.path:
        sys.path.append(_p)

import numpy as np
import ml_dtypes
from contextlib import ExitStack

import concourse.bass as bass
import concourse.bacc as bacc
import concourse.tile as tile
import concourse.mybir as mybir
from concourse.bass_utils import run_bass_kernel_spmd

F32 = mybir.dt.float32
BF16 = mybir.dt.bfloat16
NPBF16 = ml_dtypes.bfloat16

B, T, C = 2, 2048, 2048
N_HEADS, N_KV_HEADS, HD = 16, 4, 128
G = N_HEADS // N_KV_HEADS  # heads per group = 4
GW = G * HD  # 512, per-group Q width / Wo row count
N_CORES = 8
TC = 512  # q-block width
NTC = T // TC  # 4
NKT = T // HD  # 16 k-tiles of 128
NCC = C // 128  # 16 contraction chunks

_prog_cache = {}


def _build_program():
    nc = bacc.Bacc(
        "TRN2",
        target_bir_lowering=False,
        debug=False,
        enable_asserts=False,
        num_devices=N_CORES,
    )

    xT = nc.dram_tensor("xT", [C, T], BF16, kind="ExternalInput").ap()
    wq = nc.dram_tensor("wq", [C, GW], BF16, kind="ExternalInput").ap()
    wk = nc.dram_tensor("wk", [C, HD], BF16, kind="ExternalInput").ap()
    wv = nc.dram_tensor("wv", [C, HD], BF16, kind="ExternalInput").ap()
    wo = nc.dram_tensor("wo", [GW, C], BF16, kind="ExternalInput").ap()
    cos = nc.dram_tensor("cos", [HD, T], BF16, kind="ExternalInput").ap()
    sin = nc.dram_tensor("sin", [HD, T], BF16, kind="ExternalInput").ap()
    tri = nc.dram_tensor("tri", [128, 128], BF16, kind="ExternalInput").ap()
    ones = nc.dram_tensor("ones", [128, 128], BF16, kind="ExternalInput").ap()
    y = nc.dram_tensor("y", [T, C], F32, kind="ExternalOutput").ap()

    with tile.TileContext(nc) as tc, ExitStack() as ctx:
        cpool = ctx.enter_context(tc.tile_pool(name="const", bufs=1))
        big = ctx.enter_context(tc.tile_pool(name="big", bufs=1))
        xin = ctx.enter_context(tc.tile_pool(name="xin", bufs=NCC))
        rp = ctx.enter_context(tc.tile_pool(name="rp", bufs=3))
        ptp = ctx.enter_context(tc.tile_pool(name="pt", bufs=6))
        nrm = ctx.enter_context(tc.tile_pool(name="nrm", bufs=2))
        otq = ctx.enter_context(tc.tile_pool(name="otq", bufs=2))
        ysb = ctx.enter_context(tc.tile_pool(name="ysb", bufs=3))

        aps = ctx.enter_context(tc.tile_pool(name="aps", bufs=3, space="PSUM"))
        stp = ctx.enter_context(tc.tile_pool(name="stp", bufs=3, space="PSUM"))
        sbp = ctx.enter_context(tc.tile_pool(name="sbp", bufs=1, space="PSUM"))
        otp = ctx.enter_context(tc.tile_pool(name="otp", bufs=1, space="PSUM"))

        # ------------- constants / weights -------------
        wq_sb = cpool.tile([128, NCC * GW], BF16)  # [c-chunk p, ci*512 + j*128+d]
        wk_sb = cpool.tile([128, NCC * HD], BF16)
        wv_sb = cpool.tile([128, NCC * HD], BF16)
        wo_sb = cpool.tile([128, G * C], BF16)  # [row-in-head-chunk, h*C + c]
        cos_sb = cpool.tile([HD, T], BF16)
        sin_sb = cpool.tile([HD, T], BF16)
        tri_sb = cpool.tile([128, 128], BF16)
        ones_sb = cpool.tile([128, 128], BF16)

        # big activations: QT [d, h*T + t], KT [d, t], V [t-part, kt*HD + d]
        qt_sb = big.tile([128, G * T], BF16)
        kt_sb = big.tile([128, T], BF16)
        v_sb = big.tile([128, NKT * HD], BF16)

        # x chunks: whole xT resident, one tile per c-chunk
        x_sb = [xin.tile([128, T], BF16, tag="x", name=f"x{ci}") for ci in range(NCC)]

        # -- prefetch DMAs (per c-chunk: a [128, w] DRAM slab maps onto the
        # 128 partitions directly; batching chunks braids the layout) --
        def load_w_chunk(ci):
            nc.sync.dma_start(
                wq_sb[:, ci * GW : (ci + 1) * GW], wq[ci * 128 : (ci + 1) * 128, :]
            )
            nc.sync.dma_start(
                wk_sb[:, ci * HD : (ci + 1) * HD], wk[ci * 128 : (ci + 1) * 128, :]
            )
            nc.sync.dma_start(
                wv_sb[:, ci * HD : (ci + 1) * HD], wv[ci * 128 : (ci + 1) * 128, :]
            )

        load_w_chunk(0)
        nc.gpsimd.dma_start(x_sb[0][:], xT[0:128, :])
        load_w_chunk(1)
        nc.gpsimd.dma_start(x_sb[1][:], xT[128:256, :])
        for ci in range(2, NCC):
            load_w_chunk(ci)
            nc.gpsimd.dma_start(x_sb[ci][:], xT[ci * 128 : (ci + 1) * 128, :])
        nc.sync.dma_start(cos_sb[:], cos[:])
        nc.sync.dma_start(sin_sb[:], sin[:])
        nc.sync.dma_start(tri_sb[:], tri[:])
        nc.sync.dma_start(ones_sb[:], ones[:])
        for h in range(G):
            nc.sync.dma_start(
                wo_sb[:, h * C : (h + 1) * C], wo[h * 128 : (h + 1) * 128, :]
            )

        def wq_st(ci, j):
            return wq_sb[:, ci * GW + j * HD : ci * GW + (j + 1) * HD]

        def a_chunk(tci):
            """Projections for t-chunk tci: QT heads, KT, and V in [t,d]."""
            ts = slice(tci * TC, (tci + 1) * TC)
            # pass 1: q0, q1, k
            q01 = [
                aps.tile([128, TC], F32, tag="aps", name=f"qtps{tci}_{j}")
                for j in range(2)
            ]
            kt_ps = aps.tile([128, TC], F32, tag="aps", name=f"ktps{tci}")
            for ci in range(NCC):
                st, sp = (ci == 0), (ci == NCC - 1)
                for j in range(2):
                    nc.tensor.matmul(
                        q01[j][:], wq_st(ci, j), x_sb[ci][:, ts], start=st, stop=sp
                    )
                nc.tensor.matmul(
                    kt_ps[:],
                    wk_sb[:, ci * HD : (ci + 1) * HD],
                    x_sb[ci][:, ts],
                    start=st,
                    stop=sp,
                )
            # pass 2: q2, q3, v(direct [t,d] via x-stationary)
            q23 = [
                aps.tile([128, TC], F32, tag="aps", name=f"qtps{tci}_{j + 2}")
                for j in range(2)
            ]
            v_ps = aps.tile([128, TC], F32, tag="aps", name=f"vtps{tci}")
            for ci in range(NCC):
                st, sp = (ci == 0), (ci == NCC - 1)
                for j in range(2):
                    nc.tensor.matmul(
                        q23[j][:], wq_st(ci, j + 2), x_sb[ci][:, ts], start=st, stop=sp
                    )
                # one psum accumulation group for the whole bank: start only on
                # the very first sub-write (marks the full 2KB zero region),
                # stop only on the very last
                for s in range(TC // 128):
                    nc.tensor.matmul(
                        v_ps[:, s * HD : (s + 1) * HD],
                        x_sb[ci][:, tci * TC + s * 128 : tci * TC + (s + 1) * 128],
                        wv_sb[:, ci * HD : (ci + 1) * HD],
                        start=(st and s == 0),
                        stop=(sp and s == TC // 128 - 1),
                        skip_group_check=True,
                    )

            # rope on Q heads: out = q*cos + swap(q)*sin_signed
            qt_ps = q01 + q23
            for j in range(G):
                q_raw = rp.tile([128, TC], BF16, tag="qraw", name=f"qraw{tci}_{j}")
                nc.scalar.copy(q_raw[:], qt_ps[j][:])
                t1 = rp.tile([128, TC], BF16, tag="t1", name=f"t1_{tci}_{j}")
                nc.vector.tensor_mul(t1[:], q_raw[:], cos_sb[:, ts])
                qsw = rp.tile([128, TC], BF16, tag="qsw", name=f"qsw{tci}_{j}")
                nc.gpsimd.dma_start(qsw[0:64, :], q_raw[64:128, :])
                nc.gpsimd.dma_start(qsw[64:128, :], q_raw[0:64, :])
                t2 = rp.tile([128, TC], BF16, tag="t2", name=f"t2_{tci}_{j}")
                nc.vector.tensor_mul(t2[:], qsw[:], sin_sb[:, ts])
                nc.vector.tensor_add(
                    qt_sb[:, j * T + tci * TC : j * T + (tci + 1) * TC], t1[:], t2[:]
                )
            # rope on K
            k_raw = rp.tile([128, TC], BF16, tag="qraw", name=f"kraw{tci}")
            nc.scalar.copy(k_raw[:], kt_ps[:])
            t1k = rp.tile([128, TC], BF16, tag="t1", name=f"t1k{tci}")
            nc.vector.tensor_mul(t1k[:], k_raw[:], cos_sb[:, ts])
            ksw = rp.tile([128, TC], BF16, tag="qsw", name=f"ksw{tci}")
            nc.gpsimd.dma_start(ksw[0:64, :], k_raw[64:128, :])
            nc.gpsimd.dma_start(ksw[64:128, :], k_raw[0:64, :])
            t2k = rp.tile([128, TC], BF16, tag="t2", name=f"t2k{tci}")
            nc.vector.tensor_mul(t2k[:], ksw[:], sin_sb[:, ts])
            nc.vector.tensor_add(kt_sb[:, ts], t1k[:], t2k[:])
            # V psum -> sbuf (already [t, d])
            nc.scalar.copy(v_sb[:, tci * 4 * HD : (tci + 1) * 4 * HD], v_ps[:])

        def b_attn(qb):
            """Attention for q-block qb -> normalized ot_qb [d, h*TC + q]."""
            nkt = (qb + 1) * (TC // 128)
            ot_qb = otq.tile([128, G * TC], BF16, tag="ot", name=f"ot{qb}")
            for h in range(G):
                sb_ps = sbp.tile([128, TC], F32, tag="sb", name=f"sb{qb}_{h}")
                ot_ps = otp.tile([128, TC], F32, tag="otp", name=f"otp{qb}_{h}")
                for kt in range(nkt):
                    dj = kt - 4 * qb
                    f0 = max(dj, 0) * 128  # first causally-valid column
                    st, sp = (kt == 0), (kt == nkt - 1)
                    s_t = stp.tile([128, TC], F32, tag="st", name=f"st{qb}_{kt}_{h}")
                    nc.tensor.matmul(
                        s_t[:, f0:TC],
                        kt_sb[:, kt * 128 : (kt + 1) * 128],
                        qt_sb[:, h * T + qb * TC + f0 : h * T + (qb + 1) * TC],
                        start=True,
                        stop=True,
                    )
                    pt = ptp.tile([128, TC], BF16, tag="pt", name=f"pt{qb}_{kt}_{h}")
                    nc.scalar.activation(
                        pt[:, f0:TC],
                        s_t[:, f0:TC],
                        mybir.ActivationFunctionType.Exp,
                    )
                    if dj >= 0:
                        nc.vector.tensor_mul(
                            pt[:, f0 : f0 + 128], pt[:, f0 : f0 + 128], tri_sb[:]
                        )
                    nc.tensor.matmul(
                        sb_ps[:, f0:TC], ones_sb[:], pt[:, f0:TC], start=st, stop=sp
                    )
                    nc.tensor.matmul(
                        ot_ps[:, f0:TC],
                        v_sb[:, kt * HD : (kt + 1) * HD],
                        pt[:, f0:TC],
                        start=st,
                        stop=sp,
                    )
                r_f = nrm.tile([128, TC], F32, tag="rf", name=f"rf{qb}_{h}")
                nc.vector.reciprocal_approx_fast(r_f[:], sb_ps[:])
                nc.vector.tensor_mul(
                    ot_qb[:, h * TC : (h + 1) * TC], ot_ps[:], r_f[:]
                )
            return ot_qb

        def b_outproj(qb, ot_qb):
            for tl in range(TC // 128):
                tsub = qb * (TC // 128) + tl
                for cc in range(C // TC):
                    y_ps = stp.tile([128, TC], F32, tag="st", name=f"yps{tsub}_{cc}")
                    for h in range(G):
                        nc.tensor.matmul(
                            y_ps[:],
                            ot_qb[:, h * TC + tl * 128 : h * TC + (tl + 1) * 128],
                            wo_sb[:, h * C + cc * TC : h * C + (cc + 1) * TC],
                            start=(h == 0),
                            stop=(h == G - 1),
                        )
                    y_t = ysb.tile([128, TC], F32, tag="ysb", name=f"ysb{tsub}_{cc}")
                    nc.vector.tensor_copy(y_t[:], y_ps[:])
                    nc.sync.dma_start(
                        y[tsub * 128 : (tsub + 1) * 128, cc * TC : (cc + 1) * TC],
                        y_t[:],
                    )

        # ---- interleaved schedule: PE stays dense, deps always one block ahead
        a_chunk(0)
        a_chunk(1)
        ot0 = b_attn(0)
        a_chunk(2)
        b_outproj(0, ot0)
        ot1 = b_attn(1)
        a_chunk(3)
        b_outproj(1, ot1)
        ot2 = b_attn(2)
        ot3 = b_attn(3)
        b_outproj(2, ot2)
        b_outproj(3, ot3)

    nc.compile()
    return nc


def _rope_tables():
    theta = 1.0 / (10000.0 ** (np.arange(0, HD, 2, dtype=np.float32) / HD))
    freqs = np.arange(T, dtype=np.float32)[:, None] * theta[None, :]  # [T, 64]
    cos = np.concatenate([np.cos(freqs), np.cos(freqs)], axis=-1)  # [T, 128]
    sin = np.concatenate([np.sin(freqs), np.sin(freqs)], axis=-1)
    cosT = np.ascontiguousarray(cos.T).astype(np.float32)  # [128, T]
    sinT = np.ascontiguousarray(sin.T).astype(np.float32)
    sign = np.where(np.arange(HD) < HD // 2, np.float32(-1.0), np.float32(1.0))[:, None]
    sinT_signed = (sinT * sign).astype(np.float32)
    return cosT.astype(NPBF16), sinT_signed.astype(NPBF16)


def make_in_maps(x, Wq, Wk, Wv, Wo):
    x = np.asarray(x, dtype=np.float32)
    Wq = np.asarray(Wq, dtype=np.float32)
    Wk = np.asarray(Wk, dtype=np.float32)
    Wv = np.asarray(Wv, dtype=np.float32)
    Wo = np.asarray(Wo, dtype=np.float32)

    cosT, sinT = _rope_tables()
    qscale = np.float32(1.0 / np.sqrt(HD))
    p = np.arange(128)[:, None]
    f = np.arange(128)[None, :]
    tri = (p <= f).astype(NPBF16)
    ones = np.ones((128, 128), dtype=NPBF16)

    xb = [np.ascontiguousarray(x[b].T).astype(NPBF16) for b in range(B)]
    wqb = (Wq * qscale).astype(NPBF16)
    wkb = Wk.astype(NPBF16)
    wvb = Wv.astype(NPBF16)
    wob = Wo.astype(NPBF16)

    in_maps = []
    for c in range(N_CORES):
        b, g = divmod(c, N_KV_HEADS)
        in_maps.append(
            {
                "xT": xb[b],
                "wq": np.ascontiguousarray(wqb[:, g * GW : (g + 1) * GW]),
                "wk": np.ascontiguousarray(wkb[:, g * HD : (g + 1) * HD]),
                "wv": np.ascontiguousarray(wvb[:, g * HD : (g + 1) * HD]),
                "wo": np.ascontiguousarray(wob[g * GW : (g + 1) * GW, :]),
                "cos": cosT,
                "sin": sinT,
                "tri": tri,
                "ones": ones,
            }
        )
    return in_maps


def kernel(x, Wq, Wk, Wv, Wo):
    if "nc" not in _prog_cache:
        _prog_cache["nc"] = _build_program()
    nc = _prog_cache["nc"]

    in_maps = make_in_maps(x, Wq, Wk, Wv, Wo)
    res = run_bass_kernel_spmd(nc, in_maps, list(range(N_CORES)))
    _prog_cache["last_results"] = res

    out = np.zeros((B, T, C), dtype=np.float32)
    for c in range(N_CORES):
        b = c // N_KV_HEADS
        out[b] += res.results[c]["y"]
    return out


# revision 8
# speedup vs baseline: 1.3808x; 1.0225x over previous
"""Causal self-attention (GQA + RoPE) Trainium2 kernel, bf16 tensor-core path.

Full-input contract: kernel(**inputs) takes the unsharded tensors and returns
the full [B, T, C] output. Internally shards over 8 NeuronCores as
(batch b in {0,1}) x (kv-head group g in {0..3}); each core computes the
attention output of its 4 query heads (one kv head) for its batch and the
partial out-projection against its 512 rows of Wo. The host sums the 4 group
partials per batch.

v2 design (vs the fp32r baseline):
  - all matmul operands bf16 (fp32 PSUM accumulation). Host converts inputs.
  - phase A (projections) and phase B (attention q-blocks) are interleaved
    chunk-wise in emission order so the PE never idles long enough to drop
    out of its ramped p-state: A0 A1 B0a A2 B0o B1a A3 B1o B2a B3a B2o B3o.
  - PSUM budget kept at 8 banks at every point: A accumulators 3 (two passes
    per t-chunk: {q0,q1,k} then {q2,q3,v}), scores/outproj 3, denom 1, attnV 1.
  - V is produced directly in [t, d] layout (x-chunk stationary, Wv moving),
    no PE transpose pass.
  - causal masking via a 0/1 lower-triangle multiply on the exp output (bf16,
    SBUF) instead of a -1e30 add on the fp32 PSUM scores; scores, exp, the
    denominator matmul and attn@V are all restricted to the causally valid
    column range [f0:TC] of each k-tile (accumulation regions only shrink
    after the full-width kt=0 tile, so partial-range PSUM accumulate is safe).
"""

import sys

for _p in ("/opt/trn_rl_repo", "/root/.axon_site/_ro/trn_rl_repo"):
    if _p not in sys.path:
        sys.path.append(_p)

import numpy as np
import ml_dtypes
from contextlib import ExitStack

import concourse.bass as bass
import concourse.bacc as bacc
import concourse.tile as tile
import concourse.mybir as mybir
from concourse.bass_utils import run_bass_kernel_spmd

F32 = mybir.dt.float32
BF16 = mybir.dt.bfloat16
NPBF16 = ml_dtypes.bfloat16

B, T, C = 2, 2048, 2048
N_HEADS, N_KV_HEADS, HD = 16, 4, 128
G = N_HEADS // N_KV_HEADS  # heads per group = 4
GW = G * HD  # 512, per-group Q width / Wo row count
N_CORES = 8
TC = 512  # q-block width
NTC = T // TC  # 4
NKT = T // HD  # 16 k-tiles of 128
NCC = C // 128  # 16 contraction chunks

_prog_cache = {}


def _build_program():
    nc = bacc.Bacc(
        "TRN2",
        target_bir_lowering=False,
        debug=False,
        enable_asserts=False,
        num_devices=N_CORES,
    )

    xT = nc.dram_tensor("xT", [C, T], BF16, kind="ExternalInput").ap()
    wq = nc.dram_tensor("wq", [C, GW], BF16, kind="ExternalInput").ap()
    wk = nc.dram_tensor("wk", [C, HD], BF16, kind="ExternalInput").ap()
    wv = nc.dram_tensor("wv", [C, HD], BF16, kind="ExternalInput").ap()
    wo = nc.dram_tensor("wo", [GW, C], BF16, kind="ExternalInput").ap()
    cos = nc.dram_tensor("cos", [HD, T], BF16, kind="ExternalInput").ap()
    sin = nc.dram_tensor("sin", [HD, T], BF16, kind="ExternalInput").ap()
    tri = nc.dram_tensor("tri", [128, 128], BF16, kind="ExternalInput").ap()
    ones = nc.dram_tensor("ones", [128, 128], BF16, kind="ExternalInput").ap()
    y = nc.dram_tensor("y", [T, C], BF16, kind="ExternalOutput").ap()

    with tile.TileContext(nc) as tc, ExitStack() as ctx:
        cpool = ctx.enter_context(tc.tile_pool(name="const", bufs=1))
        big = ctx.enter_context(tc.tile_pool(name="big", bufs=1))
        xin = ctx.enter_context(tc.tile_pool(name="xin", bufs=NCC))
        rp = ctx.enter_context(tc.tile_pool(name="rp", bufs=3))
        ptp = ctx.enter_context(tc.tile_pool(name="pt", bufs=6))
        nrm = ctx.enter_context(tc.tile_pool(name="nrm", bufs=2))
        otq = ctx.enter_context(tc.tile_pool(name="otq", bufs=2))
        ysb = ctx.enter_context(tc.tile_pool(name="ysb", bufs=3))

        aps = ctx.enter_context(tc.tile_pool(name="aps", bufs=3, space="PSUM"))
        stp = ctx.enter_context(tc.tile_pool(name="stp", bufs=3, space="PSUM"))
        sbp = ctx.enter_context(tc.tile_pool(name="sbp", bufs=1, space="PSUM"))
        otp = ctx.enter_context(tc.tile_pool(name="otp", bufs=1, space="PSUM"))

        # ------------- constants / weights -------------
        wq_sb = cpool.tile([128, NCC * GW], BF16)  # [c-chunk p, ci*512 + j*128+d]
        wk_sb = cpool.tile([128, NCC * HD], BF16)
        wv_sb = cpool.tile([128, NCC * HD], BF16)
        wo_sb = cpool.tile([128, G * C], BF16)  # [row-in-head-chunk, h*C + c]
        cos_sb = cpool.tile([HD, T], BF16)
        sin_sb = cpool.tile([HD, T], BF16)
        tri_sb = cpool.tile([128, 128], BF16)
        ones_sb = cpool.tile([128, 128], BF16)

        # big activations: QT [d, h*T + t], KT [d, t], V [t-part, kt*HD + d]
        qt_sb = big.tile([128, G * T], BF16)
        kt_sb = big.tile([128, T], BF16)
        v_sb = big.tile([128, NKT * HD], BF16)

        # x chunks: whole xT resident, one tile per c-chunk
        x_sb = [xin.tile([128, T], BF16, tag="x", name=f"x{ci}") for ci in range(NCC)]

        # -- prefetch DMAs. Weight slabs use a rearranged access pattern so a
        # single DMA writes chunk-major layout ([128, w] per c-chunk side by
        # side); x chunks alternate between the sync and gpsimd queues so the
        # per-queue serialized transfers keep ahead of the PE.
        def chunk_major(src_ap):
            return src_ap.rearrange("(ci p) d -> p ci d", p=128)

        nc.sync.dma_start(
            wq_sb[:, 0 : 4 * GW].rearrange("p (ci d) -> p ci d", ci=4),
            chunk_major(wq[0:512, :]),
        )
        nc.gpsimd.dma_start(x_sb[0][:], xT[0:128, :])
        nc.sync.dma_start(
            wk_sb[:].rearrange("p (ci d) -> p ci d", ci=NCC), chunk_major(wk)
        )
        nc.gpsimd.dma_start(x_sb[1][:], xT[128:256, :])
        nc.sync.dma_start(
            wv_sb[:].rearrange("p (ci d) -> p ci d", ci=NCC), chunk_major(wv)
        )
        nc.gpsimd.dma_start(x_sb[2][:], xT[256:384, :])
        for q in range(1, 4):
            nc.sync.dma_start(
                wq_sb[:, q * 4 * GW : (q + 1) * 4 * GW].rearrange(
                    "p (ci d) -> p ci d", ci=4
                ),
                chunk_major(wq[q * 512 : (q + 1) * 512, :]),
            )
        for ci in range(3, NCC):
            eng = nc.gpsimd if ci % 2 else nc.sync
            eng.dma_start(x_sb[ci][:], xT[ci * 128 : (ci + 1) * 128, :])
        nc.sync.dma_start(cos_sb[:], cos[:])
        nc.gpsimd.dma_start(sin_sb[:], sin[:])
        nc.sync.dma_start(tri_sb[:], tri[:])
        nc.sync.dma_start(ones_sb[:], ones[:])
        for h in range(G):
            nc.sync.dma_start(
                wo_sb[:, h * C : (h + 1) * C], wo[h * 128 : (h + 1) * 128, :]
            )

        def wq_st(ci, j):
            return wq_sb[:, ci * GW + j * HD : ci * GW + (j + 1) * HD]

        def a_chunk(tci):
            """Projections for t-chunk tci: QT heads, KT, and V in [t,d]."""
            ts = slice(tci * TC, (tci + 1) * TC)
            # pass 1: q0, q1, k
            q01 = [
                aps.tile([128, TC], F32, tag="aps", name=f"qtps{tci}_{j}")
                for j in range(2)
            ]
            kt_ps = aps.tile([128, TC], F32, tag="aps", name=f"ktps{tci}")
            for ci in range(NCC):
                st, sp = (ci == 0), (ci == NCC - 1)
                for j in range(2):
                    nc.tensor.matmul(
                        q01[j][:], wq_st(ci, j), x_sb[ci][:, ts], start=st, stop=sp
                    )
                nc.tensor.matmul(
                    kt_ps[:],
                    wk_sb[:, ci * HD : (ci + 1) * HD],
                    x_sb[ci][:, ts],
                    start=st,
                    stop=sp,
                )
            # pass 2: q2, q3, v(direct [t,d] via x-stationary)
            q23 = [
                aps.tile([128, TC], F32, tag="aps", name=f"qtps{tci}_{j + 2}")
                for j in range(2)
            ]
            v_ps = aps.tile([128, TC], F32, tag="aps", name=f"vtps{tci}")
            for ci in range(NCC):
                st, sp = (ci == 0), (ci == NCC - 1)
                for j in range(2):
                    nc.tensor.matmul(
                        q23[j][:], wq_st(ci, j + 2), x_sb[ci][:, ts], start=st, stop=sp
                    )
                # one psum accumulation group for the whole bank: start only on
                # the very first sub-write (marks the full 2KB zero region),
                # stop only on the very last
                for s in range(TC // 128):
                    nc.tensor.matmul(
                        v_ps[:, s * HD : (s + 1) * HD],
                        x_sb[ci][:, tci * TC + s * 128 : tci * TC + (s + 1) * 128],
                        wv_sb[:, ci * HD : (ci + 1) * HD],
                        start=(st and s == 0),
                        stop=(sp and s == TC // 128 - 1),
                        skip_group_check=True,
                    )

            # rope on Q heads: out = q*cos + swap(q)*sin_signed
            qt_ps = q01 + q23
            for j in range(G):
                q_raw = rp.tile([128, TC], BF16, tag="qraw", name=f"qraw{tci}_{j}")
                nc.scalar.copy(q_raw[:], qt_ps[j][:])
                t1 = rp.tile([128, TC], BF16, tag="t1", name=f"t1_{tci}_{j}")
                nc.vector.tensor_mul(t1[:], q_raw[:], cos_sb[:, ts])
                qsw = rp.tile([128, TC], BF16, tag="qsw", name=f"qsw{tci}_{j}")
                nc.gpsimd.dma_start(qsw[0:64, :], q_raw[64:128, :])
                nc.gpsimd.dma_start(qsw[64:128, :], q_raw[0:64, :])
                t2 = rp.tile([128, TC], BF16, tag="t2", name=f"t2_{tci}_{j}")
                nc.vector.tensor_mul(t2[:], qsw[:], sin_sb[:, ts])
                nc.vector.tensor_add(
                    qt_sb[:, j * T + tci * TC : j * T + (tci + 1) * TC], t1[:], t2[:]
                )
            # rope on K
            k_raw = rp.tile([128, TC], BF16, tag="qraw", name=f"kraw{tci}")
            nc.scalar.copy(k_raw[:], kt_ps[:])
            t1k = rp.tile([128, TC], BF16, tag="t1", name=f"t1k{tci}")
            nc.vector.tensor_mul(t1k[:], k_raw[:], cos_sb[:, ts])
            ksw = rp.tile([128, TC], BF16, tag="qsw", name=f"ksw{tci}")
            nc.gpsimd.dma_start(ksw[0:64, :], k_raw[64:128, :])
            nc.gpsimd.dma_start(ksw[64:128, :], k_raw[0:64, :])
            t2k = rp.tile([128, TC], BF16, tag="t2", name=f"t2k{tci}")
            nc.vector.tensor_mul(t2k[:], ksw[:], sin_sb[:, ts])
            nc.vector.tensor_add(kt_sb[:, ts], t1k[:], t2k[:])
            # V psum -> sbuf (already [t, d])
            nc.scalar.copy(v_sb[:, tci * 4 * HD : (tci + 1) * 4 * HD], v_ps[:])

        def b_attn(qb):
            """Attention for q-block qb -> normalized ot_qb [d, h*TC + q]."""
            nkt = (qb + 1) * (TC // 128)
            ot_qb = otq.tile([128, G * TC], BF16, tag="ot", name=f"ot{qb}")
            for h in range(G):
                sb_ps = sbp.tile([128, TC], F32, tag="sb", name=f"sb{qb}_{h}")
                ot_ps = otp.tile([128, TC], F32, tag="otp", name=f"otp{qb}_{h}")
                for kt in range(nkt):
                    dj = kt - 4 * qb
                    f0 = max(dj, 0) * 128  # first causally-valid column
                    st, sp = (kt == 0), (kt == nkt - 1)
                    s_t = stp.tile([128, TC], F32, tag="st", name=f"st{qb}_{kt}_{h}")
                    nc.tensor.matmul(
                        s_t[:, f0:TC],
                        kt_sb[:, kt * 128 : (kt + 1) * 128],
                        qt_sb[:, h * T + qb * TC + f0 : h * T + (qb + 1) * TC],
                        start=True,
                        stop=True,
                    )
                    pt = ptp.tile([128, TC], BF16, tag="pt", name=f"pt{qb}_{kt}_{h}")
                    nc.scalar.activation(
                        pt[:, f0:TC],
                        s_t[:, f0:TC],
                        mybir.ActivationFunctionType.Exp,
                    )
                    if dj >= 0:
                        nc.vector.tensor_mul(
                            pt[:, f0 : f0 + 128], pt[:, f0 : f0 + 128], tri_sb[:]
                        )
                    nc.tensor.matmul(
                        sb_ps[:, f0:TC], ones_sb[:], pt[:, f0:TC], start=st, stop=sp
                    )
                    nc.tensor.matmul(
                        ot_ps[:, f0:TC],
                        v_sb[:, kt * HD : (kt + 1) * HD],
                        pt[:, f0:TC],
                        start=st,
                        stop=sp,
                    )
                r_f = nrm.tile([128, TC], F32, tag="rf", name=f"rf{qb}_{h}")
                nc.vector.reciprocal_approx_fast(r_f[:], sb_ps[:])
                nc.vector.tensor_mul(
                    ot_qb[:, h * TC : (h + 1) * TC], ot_ps[:], r_f[:]
                )
            return ot_qb

        def b_outproj(qb, ot_qb):
            for tl in range(TC // 128):
                tsub = qb * (TC // 128) + tl
                for cc in range(C // TC):
                    y_ps = stp.tile([128, TC], F32, tag="st", name=f"yps{tsub}_{cc}")
                    for h in range(G):
                        nc.tensor.matmul(
                            y_ps[:],
                            ot_qb[:, h * TC + tl * 128 : h * TC + (tl + 1) * 128],
                            wo_sb[:, h * C + cc * TC : h * C + (cc + 1) * TC],
                            start=(h == 0),
                            stop=(h == G - 1),
                        )
                    y_t = ysb.tile([128, TC], BF16, tag="ysb", name=f"ysb{tsub}_{cc}")
                    # alternate copy engine and DMA queue so neither the DVE
                    # nor a single DMA ring paces the out-projection stream
                    if cc % 2:
                        nc.scalar.copy(y_t[:], y_ps[:])
                    else:
                        nc.vector.tensor_copy(y_t[:], y_ps[:])
                    deng = nc.gpsimd if cc % 2 else nc.sync
                    deng.dma_start(
                        y[tsub * 128 : (tsub + 1) * 128, cc * TC : (cc + 1) * TC],
                        y_t[:],
                    )

        # ---- interleaved schedule: PE stays dense, deps always one block ahead
        a_chunk(0)
        a_chunk(1)
        ot0 = b_attn(0)
        a_chunk(2)
        b_outproj(0, ot0)
        ot1 = b_attn(1)
        a_chunk(3)
        b_outproj(1, ot1)
        ot2 = b_attn(2)
        ot3 = b_attn(3)
        b_outproj(2, ot2)
        b_outproj(3, ot3)

    nc.compile()
    return nc


def _rope_tables():
    theta = 1.0 / (10000.0 ** (np.arange(0, HD, 2, dtype=np.float32) / HD))
    freqs = np.arange(T, dtype=np.float32)[:, None] * theta[None, :]  # [T, 64]
    cos = np.concatenate([np.cos(freqs), np.cos(freqs)], axis=-1)  # [T, 128]
    sin = np.concatenate([np.sin(freqs), np.sin(freqs)], axis=-1)
    cosT = np.ascontiguousarray(cos.T).astype(np.float32)  # [128, T]
    sinT = np.ascontiguousarray(sin.T).astype(np.float32)
    sign = np.where(np.arange(HD) < HD // 2, np.float32(-1.0), np.float32(1.0))[:, None]
    sinT_signed = (sinT * sign).astype(np.float32)
    return cosT.astype(NPBF16), sinT_signed.astype(NPBF16)


def make_in_maps(x, Wq, Wk, Wv, Wo):
    x = np.asarray(x, dtype=np.float32)
    Wq = np.asarray(Wq, dtype=np.float32)
    Wk = np.asarray(Wk, dtype=np.float32)
    Wv = np.asarray(Wv, dtype=np.float32)
    Wo = np.asarray(Wo, dtype=np.float32)

    cosT, sinT = _rope_tables()
    qscale = np.float32(1.0 / np.sqrt(HD))
    p = np.arange(128)[:, None]
    f = np.arange(128)[None, :]
    tri = (p <= f).astype(NPBF16)
    ones = np.ones((128, 128), dtype=NPBF16)

    xb = [np.ascontiguousarray(x[b].T).astype(NPBF16) for b in range(B)]
    wqb = (Wq * qscale).astype(NPBF16)
    wkb = Wk.astype(NPBF16)
    wvb = Wv.astype(NPBF16)
    wob = Wo.astype(NPBF16)

    in_maps = []
    for c in range(N_CORES):
        b, g = divmod(c, N_KV_HEADS)
        in_maps.append(
            {
                "xT": xb[b],
                "wq": np.ascontiguousarray(wqb[:, g * GW : (g + 1) * GW]),
                "wk": np.ascontiguousarray(wkb[:, g * HD : (g + 1) * HD]),
                "wv": np.ascontiguousarray(wvb[:, g * HD : (g + 1) * HD]),
                "wo": np.ascontiguousarray(wob[g * GW : (g + 1) * GW, :]),
                "cos": cosT,
                "sin": sinT,
                "tri": tri,
                "ones": ones,
            }
        )
    return in_maps


def kernel(x, Wq, Wk, Wv, Wo):
    if "nc" not in _prog_cache:
        _prog_cache["nc"] = _build_program()
    nc = _prog_cache["nc"]

    in_maps = make_in_maps(x, Wq, Wk, Wv, Wo)
    res = run_bass_kernel_spmd(nc, in_maps, list(range(N_CORES)))
    _prog_cache["last_results"] = res

    out = np.zeros((B, T, C), dtype=np.float32)
    for c in range(N_CORES):
        b = c // N_KV_HEADS
        out[b] += res.results[c]["y"].astype(np.float32)
    return out
